# revision 16
# baseline (speedup 1.0000x reference)
"""GATNet (3-layer GAT + final linear) on 8 Trainium2 NeuronCores via Bass.

Graph/data-parallel layout (per sharding hint):
  - Nodes sharded by dst across 8 cores (6250/core).  Every core keeps a full
    replica of hA_l = [h_l | alpha_src_l] (bf16, rows padded to 256B stride)
    in DRAM; per-edge features are fetched with batched GPSIMD dma_gather
    (int16 indices, table split at row 32768 into lo/hi halves).
  - Per core, edges are grouped into B blocks (<=128 dst nodes each,
    <=LO_CAP lo-edges + <=HI_CAP hi-edges, fixed CH=LO+HI chunks of 128 edge
    slots).  Host builds one-hot S [e x n] and S^T [n x e] per chunk.
    Aggregation = PE matmul psum[n,:] += S_j^T @ msg_j, where
    msg = [h[src]*exp(e) | exp(e)] so the same matmul yields the softmax
    denominator; normalization happens after aggregation (linearity).
  - alpha_dst broadcast to edges via S^T-matmul with the block's alpha_dst
    rows (fetched batched from the x-tables).
  - Layer transition: x_l(shard) -> PE transpose -> matmul with
    Wa = [W | W@a_src | W@a_dst]; AllGather(shard) -> full hA replica.
    Layer-1 hA is computed from the replicated input directly (no exchange).
  - Final linear+sigmoid fused into layer-3 epilogue; host concatenates the
    per-core [6250,1] output shards.
"""

import os
import numpy as np
import ml_dtypes

from concourse import bass, mybir, bacc
import concourse.tile as tile
from concourse import bass_utils
from concourse.masks import make_identity

BF16 = ml_dtypes.bfloat16
NEG_SLOPE = 0.2
EPS = 1e-16
I16_SPLIT = 32768


def rup(x, m):
    return (x + m - 1) // m * m


# ---------------------------------------------------------------- config ----


class Cfg:
    def __init__(self, N, ncores, layers, lo_chunks, hi_chunks):
        self.N = N
        self.ncores = ncores
        self.shard = N // ncores
        assert self.shard * ncores == N
        self.layers = layers                       # [(Fin, H, C)]
        self.loch, self.hich = lo_chunks, hi_chunks
        self.chunks = lo_chunks + hi_chunks
        self.lo_cap = lo_chunks * 128
        self.hi_cap = hi_chunks * 128
        self.losplit = I16_SPLIT if N > I16_SPLIT else N // 2
        self.Fs = [H * C for (_, H, C) in layers]
        self.Hs = [H for (_, H, C) in layers]
        self.rowws = [F + H for F, H in zip(self.Fs, self.Hs)]
        self.rowps = [rup(r, 128) for r in self.rowws]      # padded hA rows
        # x tables: [x_l | alpha_dst_{l+1}]; x0 table holds the input x
        self.xrows = [layers[0][0] + self.Hs[0],
                      self.Fs[0] + self.Hs[1], self.Fs[1] + self.Hs[2]]
        self.xrowps = [rup(r, 128) for r in self.xrows]


REAL_CFG = Cfg(50000, 8, [(16, 8, 32), (256, 8, 32), (256, 12, 64)], 8, 4)


# ---------------------------------------------------------- host planning ----


def wrap16(vals, cap):
    """int16 idx stream -> wrapped [128, cap//16] layout (16 partitions x 8)."""
    assert len(vals) == cap and cap % 16 == 0
    a = np.asarray(vals, np.int16).reshape(cap // 16, 16).T   # [16, cap/16]
    return np.tile(a, (8, 1))                                  # [128, cap/16]


def make_plan(cfg, edge_index):
    N, shard, CH = cfg.N, cfg.shard, cfg.chunks
    src = np.concatenate([edge_index[0].astype(np.int64), np.arange(N)])
    dst = np.concatenate([edge_index[1].astype(np.int64), np.arange(N)])
    order = np.argsort(dst, kind="stable")
    src, dst = src[order].astype(np.int64), dst[order].astype(np.int64)

    bounds = np.searchsorted(dst, np.arange(0, N + 1, shard))
    is_lo = src < cfg.losplit
    deg_lo = np.bincount(dst[is_lo], minlength=N)
    deg_hi = np.bincount(dst[~is_lo], minlength=N)

    # greedy per-core blocks
    per_core_blocks = []
    for c in range(cfg.ncores):
        blocks, n = [], 0
        while n < shard:
            n_end, lo, hi = n, 0, 0
            while n_end < shard and n_end - n < 128:
                g = c * shard + n_end
                if lo + deg_lo[g] > cfg.lo_cap or hi + deg_hi[g] > cfg.hi_cap:
                    assert n_end > n, "single node exceeds caps"
                    break
                lo += deg_lo[g]
                hi += deg_hi[g]
                n_end += 1
            blocks.append((n, n_end))
            n = n_end
        per_core_blocks.append(blocks)
    B = max(len(b) for b in per_core_blocks)

    plan = {"B": B, "cores": []}
    for c in range(cfg.ncores):
        blocks = per_core_blocks[c] + \
            [(shard, shard)] * (B - len(per_core_blocks[c]))
        e0, e1 = bounds[c], bounds[c + 1]
        csrc = src[e0:e1]
        cdstl = dst[e0:e1] - c * shard
        node_starts = np.searchsorted(cdstl, np.arange(shard + 1))

        colw = CH * 128 // 16
        idx16 = np.zeros((128, B * colw), np.int16)
        nd16 = np.zeros((128, B * 8), np.int16)
        nodeidx = np.full((128, B), shard, np.int32)
        S = np.zeros((128, B * CH, 128), BF16)
        ST = np.zeros((128, B * CH, 128), BF16)
        for b, (n0, n1) in enumerate(blocks):
            nn = n1 - n0
            if nn > 0:
                nodeidx[:nn, b] = np.arange(n0, n1)
            ndstream = np.full(128, shard, np.int64)
            ndstream[:nn] = np.arange(n0, n1)
            nd16[:, b * 8:(b + 1) * 8] = wrap16(ndstream, 128)
            es, ee = node_starts[n0], node_starts[n1]
            bsrc, bdstl = csrc[es:ee], cdstl[es:ee]
            blo = bsrc < cfg.losplit
            lo_src, lo_dst = bsrc[blo], bdstl[blo]
            hi_src, hi_dst = bsrc[~blo] - cfg.losplit, bdstl[~blo]
            assert len(lo_src) <= cfg.lo_cap and len(hi_src) <= cfg.hi_cap
            lo_stream = np.zeros(cfg.lo_cap, np.int64)
            lo_stream[:len(lo_src)] = lo_src
            hi_stream = np.zeros(cfg.hi_cap, np.int64)
            hi_stream[:len(hi_src)] = hi_src
            idx16[:, b * colw: b * colw + cfg.lo_cap // 16] = \
                wrap16(lo_stream, cfg.lo_cap)
            idx16[:, b * colw + cfg.lo_cap // 16:(b + 1) * colw] = \
                wrap16(hi_stream, cfg.hi_cap)
            # S / S^T: slot i -> (partition i%128, chunk i//128)
            for sdst, base in [(lo_dst, 0), (hi_dst, cfg.lo_cap)]:
                ne = len(sdst)
                if ne == 0:
                    continue
                i = base + np.arange(ne)
                p, ch = i % 128, i // 128
                nl = (sdst - n0).astype(np.int64)
                S[p, b * CH + ch, nl] = 1.0
                ST[nl, b * CH + ch, p] = 1.0
        plan["cores"].append(
            dict(idx16=idx16, nd16=nd16, nodeidx=nodeidx,
                 S=S.reshape(128, -1), ST=ST.reshape(128, -1)))
    return plan


def fold_weights(W, a_s, a_d, H, C):
    F = H * C
    Wr = np.asarray(W, np.float32).reshape(-1, H, C)
    ws = np.einsum("fhc,hc->fh", Wr, np.asarray(a_s, np.float32))
    wd = np.einsum("fhc,hc->fh", Wr, np.asarray(a_d, np.float32))
    return np.concatenate([Wr.reshape(Wr.shape[0], -1), ws, wd], axis=1)


def seg_split(total):
    segs, o = [], 0
    while o < total:
        w = min(512, total - o)
        segs.append((o, w))
        o += w
    return segs


# ------------------------------------------------------------ bass program ----


def build_nc(cfg, B):
    CH, N, shard = cfg.chunks, cfg.N, cfg.shard
    LOCH = cfg.loch
    dt = mybir.dt
    f32, bf16, i16, i32 = dt.float32, dt.bfloat16, dt.int16, dt.int32
    colw = CH * 128 // 16
    Bh = 8                     # blocks per fetch segment (<=1024 idxs/call)

    nc = bacc.Bacc("TRN2", target_bir_lowering=False, debug=False,
                   enable_asserts=False, num_devices=cfg.ncores)

    # ---- I/O ----
    Fin1 = cfg.layers[0][0]
    FTOT = sum(cfg.Fs)
    xT = nc.dram_tensor("xT", [Fin1, N], bf16, kind="ExternalInput")
    x0tab_in = nc.dram_tensor("x0tab", [shard + 6, cfg.xrowps[0]], bf16,
                              kind="ExternalInput")
    Was = [nc.dram_tensor(f"Wa{li}", [cfg.layers[li][0],
                                      cfg.Fs[li] + 2 * cfg.Hs[li]], bf16,
                          kind="ExternalInput") for li in range(3)]
    breps = [nc.dram_tensor(f"brep{li}", [128, cfg.Fs[li]], f32,
                            kind="ExternalInput") for li in range(3)]
    wf_rep = nc.dram_tensor("wf_rep", [128, FTOT], f32, kind="ExternalInput")
    bf_sc = nc.dram_tensor("bf_sc", [128, 1], f32, kind="ExternalInput")
    idx16_in = nc.dram_tensor("idx16", [128, B * colw], i16,
                              kind="ExternalInput")
    nd16_in = nc.dram_tensor("nd16", [128, B * 8], i16, kind="ExternalInput")
    nodeidx_in = nc.dram_tensor("nodeidx", [128, B], i32, kind="ExternalInput")
    S_in = nc.dram_tensor("S", [128, B * CH * 128], bf16, kind="ExternalInput")
    ST_in = nc.dram_tensor("ST", [128, B * CH * 128], bf16,
                           kind="ExternalInput")
    out = nc.dram_tensor("out", [shard, 1], f32, kind="ExternalOutput")

    # ---- internal DRAM ----
    hA_full = [nc.dram_tensor(f"hAfull{li}", [N, cfg.rowps[li]], bf16,
                              kind="Internal") for li in range(3)]
    hA_shard = [None] + [nc.dram_tensor(f"hAshard{li}",
                                        [shard, cfg.rowps[li]],
                                        bf16, kind="Internal")
                         for li in (1, 2)]
    xtab = [x0tab_in] + [nc.dram_tensor(f"xtab{li}",
                                        [shard + 6, cfg.xrowps[li]],
                                        bf16, kind="Internal")
                         for li in (1, 2)]
    out_buf = nc.dram_tensor("out_buf", [shard + 1, 1], f32, kind="Internal")

    with tile.TileContext(nc) as tc:
        with tc.tile_pool(name="const", bufs=1) as cpool, \
             tc.tile_pool(name="io", bufs=3) as iop, \
             tc.tile_pool(name="gath", bufs=2) as gp, \
             tc.tile_pool(name="fetch", bufs=1) as fp, \
             tc.tile_pool(name="work", bufs=2) as wp, \
             tc.tile_pool(name="small", bufs=3) as sp, \
             tc.tile_pool(name="psum", bufs=2, space="PSUM") as pp:

            ident = cpool.tile([128, 128], bf16)
            make_identity(nc, ident[:])
            wa_sb = []
            for li in range(3):
                Fin, H = cfg.layers[li][0], cfg.Hs[li]
                tiles = []
                for f0 in range(0, Fin, 128):
                    w = min(128, Fin - f0)
                    t = cpool.tile([128, cfg.Fs[li] + 2 * H], bf16,
                                   tag=f"wa{li}_{f0}", name=f"wa{li}_{f0}")
                    nc.sync.dma_start(out=t[:w], in_=Was[li][f0:f0 + w, :])
                    tiles.append((t, w))
                wa_sb.append(tiles)
            brep_sb = []
            for li in range(3):
                t = cpool.tile([128, cfg.Fs[li]], f32, tag=f"brep{li}",
                               name=f"brepsb{li}")
                nc.sync.dma_start(out=t[:], in_=breps[li][:, :])
                brep_sb.append(t)
            wf_sb = cpool.tile([128, FTOT], f32)
            nc.sync.dma_start(out=wf_sb[:], in_=wf_rep[:, :])
            bf_sb = cpool.tile([128, 1], f32)
            nc.sync.dma_start(out=bf_sb[:], in_=bf_sc[:, :])
            idx_sb = cpool.tile([128, B * colw], i16)
            nc.sync.dma_start(out=idx_sb[:], in_=idx16_in[:, :])
            nd_sb = cpool.tile([128, B * 8], i16)
            nc.sync.dma_start(out=nd_sb[:], in_=nd16_in[:, :])
            nidx_sb = cpool.tile([128, B], i32)
            nc.sync.dma_start(out=nidx_sb[:], in_=nodeidx_in[:, :])

            # zero dummy rows of internal x tables
            zrow = cpool.tile([1, 512], bf16)
            nc.vector.memset(zrow[:], 0.0)
            for li in (1, 2):
                nc.sync.dma_start(out=xtab[li][shard:shard + 1, :],
                                  in_=zrow[:1, :cfg.xrowps[li]])

            # ---------------- phase B1: hA1 = x @ Wa1 for ALL nodes ----------
            roww0 = cfg.rowws[0]
            wa1_t = wa_sb[0][0][0]
            for t0 in range(0, N, 128):
                w = min(128, N - t0)
                lhs = iop.tile([Fin1, 128], bf16, tag="b1lhs")
                if w < 128:
                    nc.vector.memset(lhs[:], 0.0)
                nc.sync.dma_start(out=lhs[:, :w], in_=xT[:, t0:t0 + w])
                ph = pp.tile([128, cfg.Fs[0] + 2 * cfg.Hs[0]], f32, tag="agg0")
                nc.tensor.matmul(out=ph[:], lhsT=lhs[:], rhs=wa1_t[:Fin1],
                                 start=True, stop=True)
                hcp = iop.tile([128, roww0], bf16, tag="b1h")
                nc.vector.tensor_copy(out=hcp[:w], in_=ph[:w, :roww0])
                nc.sync.dma_start(out=hA_full[0][t0:t0 + w, :roww0],
                                  in_=hcp[:w])

            # ---------------- layers ----------------------------------------
            _maxl = int(os.environ.get("GAT_LAYERS", "3"))
            for li in range(_maxl):
                Fin, H, C = cfg.layers[li]
                F, rowp = cfg.Fs[li], cfg.rowps[li]
                segs = seg_split(F + H)
                is_last = li == 2
                adcol = Fin if li == 0 else cfg.Fs[li - 1]   # alpha_dst col
                xrowp = cfg.xrowps[li]

                for half in range((B + Bh - 1) // Bh):
                    b0 = half * Bh
                    nb = min(Bh, B - b0)
                    if nb <= 0:
                        continue
                    xad = fp.tile([128, Bh, xrowp], bf16, tag="xad")
                    nc.gpsimd.dma_gather(
                        xad[:, :nb, :], xtab[li][:],
                        nd_sb[:, b0 * 8:(b0 + nb) * 8],
                        nb * 128, nb * 128, xrowp)
                    if is_last:
                        x1g = fp.tile([128, Bh, cfg.xrowps[1]], bf16,
                                      tag="x1g")
                        nc.gpsimd.dma_gather(
                            x1g[:, :nb, :], xtab[1][:],
                            nd_sb[:, b0 * 8:(b0 + nb) * 8],
                            nb * 128, nb * 128, cfg.xrowps[1])

                    for b in range(b0, b0 + nb):
                        br = b - b0
                        hg = gp.tile([128, CH, rowp], bf16, tag="hg")
                        if li == 0 and b < 2:
                            nc.vector.memset(hg[:], 0.0)
                        nc.gpsimd.dma_gather(
                            hg[:, :LOCH, :], hA_full[li][0:cfg.losplit, :],
                            idx_sb[:, b * colw: b * colw + cfg.lo_cap // 16],
                            cfg.lo_cap, cfg.lo_cap, rowp)
                        nc.gpsimd.dma_gather(
                            hg[:, LOCH:, :], hA_full[li][cfg.losplit:N, :],
                            idx_sb[:, b * colw + cfg.lo_cap // 16:
                                   (b + 1) * colw],
                            cfg.hi_cap, cfg.hi_cap, rowp)
                        S_sb = wp.tile([128, CH * 128], bf16, tag="S")
                        nc.sync.dma_start(
                            out=S_sb[:],
                            in_=S_in[:, b * CH * 128:(b + 1) * CH * 128])
                        ST_sb = wp.tile([128, CH * 128], bf16, tag="ST")
                        nc.sync.dma_start(
                            out=ST_sb[:],
                            in_=ST_in[:, b * CH * 128:(b + 1) * CH * 128])

                        # alpha_dst broadcast to edges: S^T_j @ ad_block
                        adps = pp.tile([128, CH * H], f32, tag="adps")
                        for j in range(CH):
                            nc.tensor.matmul(
                                out=adps[:, j * H:(j + 1) * H],
                                lhsT=ST_sb[:, j * 128:(j + 1) * 128],
                                rhs=xad[:, br, adcol:adcol + H],
                                start=True, stop=True)

                        # e = lrelu(alpha_s[src] + alpha_d[dst]); ex = exp(e)
                        et = sp.tile([128, CH * H], f32, tag="et")
                        nc.vector.tensor_tensor(
                            out=et[:].rearrange("p (c h) -> p c h", c=CH),
                            in0=hg[:, :, F:F + H],
                            in1=adps[:].rearrange("p (c h) -> p c h", c=CH),
                            op=mybir.AluOpType.add)
                        et2 = sp.tile([128, CH * H], f32, tag="et2")
                        nc.vector.tensor_scalar_mul(out=et2[:], in0=et[:],
                                                    scalar1=NEG_SLOPE)
                        nc.vector.tensor_tensor(out=et[:], in0=et[:],
                                                in1=et2[:],
                                                op=mybir.AluOpType.max)
                        ex = sp.tile([128, CH * H], f32, tag="ex")
                        nc.scalar.activation(
                            out=ex[:], in_=et[:],
                            func=mybir.ActivationFunctionType.Exp)

                        # msg = [hg * ex | ex]
                        msg = gp.tile([128, CH, F + H], bf16, tag="msg")
                        ex3 = ex[:].rearrange("p (c h) -> p c h", c=CH)
                        nc.vector.tensor_tensor(
                            out=msg[:, :, :F].rearrange(
                                "p c (h k) -> p c h k", h=H),
                            in0=hg[:, :, :F].rearrange(
                                "p c (h k) -> p c h k", h=H),
                            in1=ex3.to_broadcast([128, CH, H, C]),
                            op=mybir.AluOpType.mult)
                        nc.vector.tensor_copy(out=msg[:, :, F:], in_=ex3)

                        # aggregate
                        pts = [pp.tile([128, w], f32, tag=f"agg{si}",
                                       name=f"agg{si}")
                               for si, (o, w) in enumerate(segs)]
                        for j in range(CH):
                            lhsT = S_sb[:, j * 128:(j + 1) * 128]
                            for (o, w), pt in zip(segs, pts):
                                nc.tensor.matmul(out=pt[:], lhsT=lhsT,
                                                 rhs=msg[:, j, o:o + w],
                                                 start=(j == 0),
                                                 stop=(j == CH - 1))

                        # normalize + bias + relu
                        dseg = len(segs) - 1
                        dof = F - segs[dseg][0]
                        den = sp.tile([128, H], f32, tag="den")
                        nc.vector.tensor_scalar_add(
                            out=den[:], in0=pts[dseg][:, dof:dof + H],
                            scalar1=EPS)
                        rec = sp.tile([128, H], f32, tag="rec")
                        nc.vector.reciprocal(out=rec[:], in_=den[:])
                        xt = wp.tile([128, F], f32, tag="xt")
                        for si, (o, w) in enumerate(segs):
                            wF = min(w, F - o)
                            h0, nh = o // C, min(w, F - o) // C
                            nc.vector.tensor_tensor(
                                out=xt[:, o:o + wF].rearrange(
                                    "p (h k) -> p h k", h=nh),
                                in0=pts[si][:, :wF].rearrange(
                                    "p (h k) -> p h k", h=nh),
                                in1=rec[:, h0:h0 + nh].to_broadcast(
                                    [128, nh, C]),
                                op=mybir.AluOpType.mult)
                        nc.vector.tensor_tensor(out=xt[:], in0=xt[:],
                                                in1=brep_sb[li][:],
                                                op=mybir.AluOpType.add)
                        x_bf = wp.tile([128, F], bf16, tag="xbf")
                        nc.vector.tensor_scalar_max(out=x_bf[:], in0=xt[:],
                                                    scalar1=0.0)

                        nidx = nidx_sb[:, b:b + 1]
                        if not is_last:
                            nc.gpsimd.indirect_dma_start(
                                out=xtab[li + 1][:], in_=x_bf[:],
                                in_offset=None,
                                out_offset=bass.IndirectOffsetOnAxis(
                                    ap=nidx, axis=0))
                        elif os.environ.get("GAT_NO_FINAL"):
                            res = sp.tile([128, 1], f32, tag="res")
                            nc.vector.tensor_copy(out=res[:], in_=x_bf[:, 0:1])
                            nc.gpsimd.indirect_dma_start(
                                out=out_buf[:], in_=res[:], in_offset=None,
                                out_offset=bass.IndirectOffsetOnAxis(
                                    ap=nidx, axis=0))
                        else:
                            scratch = wp.tile([128, FTOT], bf16,
                                              tag="fscratch", bufs=1)
                            F01, F02 = cfg.Fs[0], cfg.Fs[0] + cfg.Fs[1]
                            nc.vector.tensor_tensor(
                                out=scratch[:, :F01],
                                in0=x1g[:, br, 0:cfg.Fs[0]],
                                in1=wf_sb[:, :F01], op=mybir.AluOpType.mult)
                            nc.vector.tensor_tensor(
                                out=scratch[:, F01:F02],
                                in0=xad[:, br, 0:cfg.Fs[1]],
                                in1=wf_sb[:, F01:F02], op=mybir.AluOpType.mult)
                            nc.vector.tensor_tensor(
                                out=scratch[:, F02:], in0=x_bf[:],
                                in1=wf_sb[:, F02:], op=mybir.AluOpType.mult)
                            acc = sp.tile([128, 1], f32, tag="acc")
                            nc.vector.reduce_sum(out=acc[:], in_=scratch[:],
                                                 axis=mybir.AxisListType.X)
                            res = sp.tile([128, 1], f32, tag="res")
                            nc.scalar.activation(
                                out=res[:], in_=acc[:],
                                func=mybir.ActivationFunctionType.Sigmoid,
                                bias=bf_sb[:, 0:1])
                            nc.gpsimd.indirect_dma_start(
                                out=out_buf[:], in_=res[:], in_offset=None,
                                out_offset=bass.IndirectOffsetOnAxis(
                                    ap=nidx, axis=0))

                # ------- phase B(l+1) + AllGather --------------------------
                if not is_last and li + 1 < _maxl:
                    lin = li + 1
                    Fn, Hn = cfg.Fs[lin], cfg.Hs[lin]
                    rowwn = cfg.rowws[lin]
                    nsegs = seg_split(Fn + 2 * Hn)
                    for t0 in range(0, shard, 128):
                        w = min(128, shard - t0)
                        xin = iop.tile([128, F], bf16, tag="bx")
                        if w < 128:
                            nc.vector.memset(xin[:], 0.0)
                        nc.sync.dma_start(out=xin[:w],
                                          in_=xtab[li + 1][t0:t0 + w, 0:F])
                        phs = [pp.tile([128, w2], f32, tag=f"agg{si}",
                                       name=f"bh{si}")
                               for si, (o2, w2) in enumerate(nsegs)]
                        for fi in range(F // 128):
                            ptr = pp.tile([128, 128], bf16, tag="btr")
                            nc.tensor.transpose(
                                out=ptr[:],
                                in_=xin[:, fi * 128:(fi + 1) * 128],
                                identity=ident[:])
                            xTs = iop.tile([128, 128], bf16, tag="bxT")
                            nc.vector.tensor_copy(out=xTs[:], in_=ptr[:])
                            wa_t, ww = wa_sb[lin][fi]
                            for (o2, w2), ph2 in zip(nsegs, phs):
                                nc.tensor.matmul(
                                    out=ph2[:], lhsT=xTs[:],
                                    rhs=wa_t[:ww, o2:o2 + w2],
                                    start=(fi == 0),
                                    stop=(fi == F // 128 - 1))
                        hcp = iop.tile([128, rowwn], bf16, tag="bhcp")
                        for si, (o2, w2) in enumerate(nsegs):
                            wh = min(w2, rowwn - o2)
                            if wh > 0:
                                nc.vector.tensor_copy(
                                    out=hcp[:w, o2:o2 + wh],
                                    in_=phs[si][:w, :wh])
                        acp = sp.tile([128, Hn], bf16, tag="bacp")
                        dseg2 = len(nsegs) - 1
                        dof2 = rowwn - nsegs[dseg2][0]
                        nc.vector.tensor_copy(
                            out=acp[:w], in_=phs[dseg2][:w, dof2:dof2 + Hn])
                        nc.sync.dma_start(
                            out=hA_shard[lin][t0:t0 + w, :rowwn],
                            in_=hcp[:w])
                        nc.sync.dma_start(
                            out=xtab[lin][t0:t0 + w, F:F + Hn], in_=acp[:w])
                    if not os.environ.get("GAT_NO_AG"):
                        nc.gpsimd.collective_compute(
                            "AllGather", mybir.AluOpType.bypass,
                            replica_groups=[list(range(cfg.ncores))],
                            ins=[hA_shard[lin][:]], outs=[hA_full[lin][:]])

            nc.sync.dma_start(out=out[:, :], in_=out_buf[:shard, :])

    nc.compile()
    return nc


# ------------------------------------------------------------- host entry ----


def make_inputs(cfg, plan, x, W1, as1, ad1, b1, W2, as2, ad2, b2,
                W3, as3, ad3, b3, Wf, bf):
    x = np.asarray(x, np.float32)
    xT = np.ascontiguousarray(x.T).astype(BF16)
    Wa = [fold_weights(W, a_s, a_d, H, C)
          for (W, a_s, a_d, (Fin, H, C)) in
          [(W1, as1, ad1, cfg.layers[0]), (W2, as2, ad2, cfg.layers[1]),
           (W3, as3, ad3, cfg.layers[2])]]
    breps = [np.broadcast_to(np.asarray(b, np.float32)[None, :],
                             (128, len(np.asarray(b).ravel()))).copy()
             for b in (b1, b2, b3)]
    wf_rep = np.broadcast_to(np.asarray(Wf, np.float32).reshape(1, -1),
                             (128, sum(cfg.Fs))).copy()
    bf_sc = np.broadcast_to(np.asarray(bf, np.float32).reshape(1, 1),
                            (128, 1)).copy()
    shard = cfg.shard
    Fin1, H1 = cfg.layers[0][0], cfg.Hs[0]
    ad1_full = (x @ Wa[0][:, cfg.Fs[0] + H1:]).astype(BF16)   # [N, H1]
    common = {"xT": xT,
              "Wa0": Wa[0].astype(BF16), "Wa1": Wa[1].astype(BF16),
              "Wa2": Wa[2].astype(BF16),
              "brep0": breps[0], "brep1": breps[1], "brep2": breps[2],
              "wf_rep": wf_rep, "bf_sc": bf_sc}
    in_maps = []
    for c in range(cfg.ncores):
        pc = plan["cores"][c]
        x0tab = np.zeros((shard + 6, cfg.xrowps[0]), BF16)
        x0tab[:shard, :Fin1] = x[c * shard:(c + 1) * shard].astype(BF16)
        x0tab[:shard, Fin1:Fin1 + H1] = ad1_full[c * shard:(c + 1) * shard]
        m = dict(common)
        m.update(x0tab=x0tab, idx16=pc["idx16"], nd16=pc["nd16"],
                 nodeidx=pc["nodeidx"], S=pc["S"], ST=pc["ST"])
        in_maps.append(m)
    return in_maps


_CACHE = {}


def _get_compiled(cfg, edge_index):
    key = hash(np.asarray(edge_index).tobytes())
    if key not in _CACHE:
        plan = make_plan(cfg, np.asarray(edge_index))
        nc = build_nc(cfg, plan["B"])
        _CACHE.clear()
        _CACHE[key] = (plan, nc)
    return _CACHE[key]


def kernel(x, edge_index, JetRawPt, W1, as1, ad1, b1, W2, as2, ad2, b2,
           W3, as3, ad3, b3, Wf, bf):
    cfg = REAL_CFG
    plan, nc = _get_compiled(cfg, np.asarray(edge_index))
    in_maps = make_inputs(cfg, plan, x, W1, as1, ad1, b1, W2, as2, ad2, b2,
                          W3, as3, ad3, b3, Wf, bf)
    res = bass_utils.run_bass_kernel_spmd(nc, in_maps,
                                          core_ids=list(range(cfg.ncores)))
    return np.concatenate([res.results[c]["out"]
                           for c in range(cfg.ncores)], axis=0)
